# revision 4
# baseline (speedup 1.0000x reference)
"""EndPointAggregator Trainium2 kernel.

out[j] = concat(table[starts[j]], table[ends[j]], tanh((ends[j]-starts[j]) @ w.T + b))

Strategy (8 NeuronCores, data-parallel over spans):
  - each core owns 25000 spans (padded to 25600 = 25 chunks x 1024)
  - per chunk: two `dma_gather` instructions (custom SWDGE gather ucode) pull
    1024 table rows each from HBM into SBUF tiles [128, 8, 768]
  - slot order inside a chunk is permuted (span = k*1024 + p*8 + c) so the
    HWDGE write-back emits 24KB-contiguous runs per partition
  - dist_emb = tanh(w*(e-s)+b) computed once for the whole core on DVE/ACT
  - three device outputs (outS/outE/outD); host reassembles [200000, 1538]
"""

import numpy as np

import concourse.bacc as bacc
import concourse.bass as bass
import concourse.mybir as mybir
import concourse.tile as tile
from concourse.bass_utils import run_bass_kernel_spmd

N_CORES = 8
SEQ_LEN = 4096
DIM = 768
N_SPANS = 200000

N_PER_CORE = N_SPANS // N_CORES  # 25000
CHUNK = 1024                     # spans gathered per dma_gather instruction
CPP = CHUNK // 128               # free-dim cols per partition per chunk (8)
NCH = -(-N_PER_CORE // CHUNK)    # 25 chunks
NPAD = NCH * CHUNK               # 25600
PERP = NPAD // 128               # spans per partition for dist layout (200)
IDXC = CHUNK // 16               # idx cols per chunk in wrapped layout (64)

F32 = mybir.dt.float32
I32 = mybir.dt.int32
I16 = mybir.dt.int16


def build_module(nch=NCH, trace_sim=False):
    """Build the per-core Bass module (same NEFF on all 8 cores)."""
    npad = nch * CHUNK
    perp = npad // 128
    nc = bacc.Bacc(
        "TRN2",
        target_bir_lowering=False,
        debug=False,
        num_devices=N_CORES,
    )
    table = nc.dram_tensor("table", [SEQ_LEN, DIM], F32, kind="ExternalInput").ap()
    idx_s = nc.dram_tensor("idx_s", [128, nch * IDXC], I16, kind="ExternalInput").ap()
    idx_e = nc.dram_tensor("idx_e", [128, nch * IDXC], I16, kind="ExternalInput").ap()
    s_c = nc.dram_tensor("s_c", [128, perp], I32, kind="ExternalInput").ap()
    e_c = nc.dram_tensor("e_c", [128, perp], I32, kind="ExternalInput").ap()
    wb = nc.dram_tensor("wb", [1, 4], F32, kind="ExternalInput").ap()
    outS = nc.dram_tensor("outS", [npad, DIM], F32, kind="ExternalOutput").ap()
    outE = nc.dram_tensor("outE", [npad, DIM], F32, kind="ExternalOutput").ap()
    outD = nc.dram_tensor("outD", [128, perp * 2], F32, kind="ExternalOutput").ap()

    # chunk-view of the big outputs: row = k*1024 + p*8 + c
    outS_v = outS.rearrange("(k p c) d -> k p c d", p=128, c=CPP)
    outE_v = outE.rearrange("(k p c) d -> k p c d", p=128, c=CPP)

    with tile.TileContext(nc, trace_sim=trace_sim) as tc:
        with (
            tc.tile_pool(name="const", bufs=1) as cpool,
            tc.tile_pool(name="emb", bufs=3) as epool,
        ):
            # ---- index arrays for the gathers (whole core at once) ----
            idx_s_t = cpool.tile([128, nch * IDXC], I16)
            idx_e_t = cpool.tile([128, nch * IDXC], I16)
            nc.sync.dma_start(out=idx_s_t[:], in_=idx_s)
            nc.sync.dma_start(out=idx_e_t[:], in_=idx_e)

            # ---- dist_emb chain (tiny, independent) ----
            s_t = cpool.tile([128, perp], I32)
            e_t = cpool.tile([128, perp], I32)
            nc.sync.dma_start(out=s_t[:], in_=s_c)
            nc.sync.dma_start(out=e_t[:], in_=e_c)
            wb_t = cpool.tile([128, 4], F32, tag="wb_in")
            nc.sync.dma_start(out=wb_t[:1, :], in_=wb)
            wb_bc = cpool.tile([128, 4], F32, tag="wb_bc")
            nc.gpsimd.partition_broadcast(wb_bc[:], wb_t[:1, :])

            d_i = cpool.tile([128, perp], I32)
            nc.vector.tensor_tensor(
                out=d_i[:], in0=e_t[:], in1=s_t[:], op=mybir.AluOpType.subtract
            )
            d_f = cpool.tile([128, perp], F32)
            nc.vector.tensor_copy(out=d_f[:], in_=d_i[:])

            dist = cpool.tile([128, perp, 2], F32)
            # out = tanh(d * w_k + b_k), k = 0, 1
            nc.scalar.activation(
                dist[:, :, 0],
                d_f[:],
                mybir.ActivationFunctionType.Tanh,
                bias=wb_bc[:, 2:3],
                scale=wb_bc[:, 0:1],
            )
            nc.scalar.activation(
                dist[:, :, 1],
                d_f[:],
                mybir.ActivationFunctionType.Tanh,
                bias=wb_bc[:, 3:4],
                scale=wb_bc[:, 1:2],
            )
            nc.sync.dma_start(out=outD, in_=dist[:].rearrange("p c two -> p (c two)"))

            # ---- main gather loop ----
            for k in range(nch):
                ts = epool.tile([128, CPP, DIM], F32, tag="ts")
                te = epool.tile([128, CPP, DIM], F32, tag="te")
                nc.gpsimd.dma_gather(
                    ts[:],
                    table,
                    idx_s_t[:, k * IDXC : (k + 1) * IDXC],
                    CHUNK,
                    CHUNK,
                    DIM,
                )
                nc.gpsimd.dma_gather(
                    te[:],
                    table,
                    idx_e_t[:, k * IDXC : (k + 1) * IDXC],
                    CHUNK,
                    CHUNK,
                    DIM,
                )
                nc.sync.dma_start(out=outS_v[k], in_=ts[:])
                nc.sync.dma_start(out=outE_v[k], in_=te[:])

    nc.compile()
    return nc


def _prep_core_inputs(starts, ends, dist_w, dist_b, table_f32, nch=NCH):
    """Host-side marshalling of one core's span slice into device layouts."""
    npad = nch * CHUNK
    perp = npad // 128
    n = starts.shape[0]
    sp = np.zeros(npad, np.int16)
    ep = np.zeros(npad, np.int16)
    sp[:n] = starts.astype(np.int16)
    ep[:n] = ends.astype(np.int16)

    def wrap(v):
        # slot i of chunk k holds span k*1024 + (i%128)*8 + i//128;
        # wrapped layout: idx i at (partition i%16, col i//16), replicated x8
        slots = v.reshape(nch, 128, CPP).transpose(0, 2, 1).reshape(nch, CHUNK)
        # W[p16, k*IDXC + col] = slots[k, col*16 + p16]
        w = (
            slots.reshape(nch, IDXC, 16)
            .transpose(2, 0, 1)
            .reshape(16, nch * IDXC)
        )
        return np.tile(w, (8, 1)).copy()

    sw = np.zeros(npad, np.int32)
    ew = np.zeros(npad, np.int32)
    sw[:n] = starts.astype(np.int32)
    ew[:n] = ends.astype(np.int32)

    wbv = np.array(
        [[dist_w[0, 0], dist_w[1, 0], dist_b[0], dist_b[1]]], np.float32
    )
    return {
        "table": table_f32,
        "idx_s": wrap(sp),
        "idx_e": wrap(ep),
        "s_c": sw.reshape(128, perp),
        "e_c": ew.reshape(128, perp),
        "wb": wbv,
    }


_module_cache = {}


def get_module():
    if "nc" not in _module_cache:
        _module_cache["nc"] = build_module()
    return _module_cache["nc"]


def make_in_maps(sentence_embeddings, sentence_spans, dist_w, dist_b):
    table_f32 = np.ascontiguousarray(np.asarray(sentence_embeddings, np.float32))
    spans = np.asarray(sentence_spans)
    dist_w = np.asarray(dist_w, np.float32)
    dist_b = np.asarray(dist_b, np.float32)
    starts = spans[:, 0]
    ends = spans[:, 1]
    in_maps = []
    for c in range(N_CORES):
        sl = slice(c * N_PER_CORE, (c + 1) * N_PER_CORE)
        in_maps.append(
            _prep_core_inputs(starts[sl], ends[sl], dist_w, dist_b, table_f32)
        )
    return in_maps


def run_spmd(in_maps, **kw):
    return run_bass_kernel_spmd(
        get_module(), in_maps, core_ids=list(range(N_CORES)), **kw
    )


def assemble(results):
    out = np.empty((N_SPANS, 2 * DIM + 2), np.float32)
    for c, r in enumerate(results):
        sl = slice(c * N_PER_CORE, (c + 1) * N_PER_CORE)
        out[sl, :DIM] = r["outS"][:N_PER_CORE]
        out[sl, DIM : 2 * DIM] = r["outE"][:N_PER_CORE]
        out[sl, 2 * DIM :] = r["outD"].reshape(NPAD, 2)[:N_PER_CORE]
    return out


def kernel(sentence_embeddings, sentence_spans, dist_w, dist_b):
    in_maps = make_in_maps(sentence_embeddings, sentence_spans, dist_w, dist_b)
    res = run_spmd(in_maps)
    return assemble(res.results)
